# revision 23
# baseline (speedup 1.0000x reference)
"""Trainium2 Bass kernel for pairwise-message GNN block.

reference math (B=4, N=256, F=128, MID=192):
    out[b,i] = sum_{j != i} tanh(relu(concat(x_i, x_j) @ W1 + b1) @ W2 + b2)

Decomposition used here:
    concat(x_i,x_j)@W1 = x_i@W1a + x_j@W1b          (W1a = W1[:F], W1b = W1[F:])
    out_i = sum_{j=0..N-1} tanh(relu(A_i + B_j)@W2 + b2) - tanh(relu(A_i + B_i)@W2 + b2)

Sharding: 8 cores = 4 batches x 2 halves of the source-node axis i.
Each core gets x[b].T, its own x[b, half].T, and replicated weights; computes
a [128 out_f, 128 i] transposed output tile; host reassembles + transposes.

Per-core dataflow (all matmuls bf16 with f32 PSUM accumulation):
    AT[k,i]  = (xiT.T @ W1a).T + b1      SBUF f32 [128+64, 128]
    BT[k,j]  = (xT.T @ W1b).T            SBUF bf16 [128, 256] + dup'd k1 rows
    loop over i (4 per 2-bank PSUM tile):
      HT0      = relu(BT0 + AT0[:,i])    DVE tensor_scalar add+max -> bf16 (per i)
      HT1pair  = relu(BT1dup + AT1prs)   one op covers the k=128:192 chunk of TWO i's
      PSUM     = W2a.T@HT0 + W2b.T@HT1   PE -> [128 o, 256 j] per i
      VT       = tanh(PSUM + b2)         ACT, batched FD=1024 (or 768 + one
                                         FD=256 op with accum_out: every 8th i's
                                         j-reduction rides the ACT accumulator)
      OT[:,i]  = sum_j VT                DVE tensor_scalar mult 1.0 + accum_out
    OT -= VdT (precomputed diagonal terms); DMA out.
"""

import numpy as np

B, N, F, MID, OUT = 4, 256, 128, 128 + 64, 128
NI = N // 2  # i's per core
K0, K1 = 128, MID - 128  # MID partition chunks
IB = 4  # i's per PSUM/activation block (2 PSUM banks)

_CACHE = {}


def _build():
    import concourse.bass as bass
    import concourse.tile as tile
    from concourse import bacc, mybir

    f32 = mybir.dt.float32
    bf16 = mybir.dt.bfloat16
    Alu = mybir.AluOpType
    Act = mybir.ActivationFunctionType

    nc = bacc.Bacc(
        "TRN2",
        target_bir_lowering=False,
        debug=False,
        enable_asserts=True,
        num_devices=8,
    )

    xt_d = nc.dram_tensor("xt", [F, N], f32, kind="ExternalInput").ap()
    xit_d = nc.dram_tensor("xit", [F, NI], f32, kind="ExternalInput").ap()
    # w1: [2F, MID] natural; w2/b1 zero-padded on host to 2*K0 rows so a
    # single folded-AP DMA loads both partition chunks.
    w1_d = nc.dram_tensor("w1", [2 * F, MID], f32, kind="ExternalInput").ap()
    w2_d = nc.dram_tensor("w2", [2 * K0, OUT], f32, kind="ExternalInput").ap()
    b1_d = nc.dram_tensor("b1", [2 * K0, 1], f32, kind="ExternalInput").ap()
    b2_d = nc.dram_tensor("b2", [OUT, 1], f32, kind="ExternalInput").ap()
    b2rep_d = nc.dram_tensor("b2rep", [1, IB * N], f32, kind="ExternalInput").ap()
    out_d = nc.dram_tensor("out", [OUT, NI], f32, kind="ExternalOutput").ap()

    with tile.TileContext(nc) as tc:
        with (
            tc.tile_pool(name="const", bufs=1) as const,
            tc.tile_pool(name="work", bufs=4) as work,
        ):
            # ---- load (3 DMAs per HWDGE engine) + cast ----
            xt_f = const.tile([F, N], f32)
            nc.sync.dma_start(xt_f[:], xt_d[:])
            w1_f = const.tile([F, 2, MID], f32)  # [:,0,:]=W1a  [:,1,:]=W1b
            nc.scalar.dma_start(w1_f[:], w1_d.rearrange("(two f) k -> f two k", two=2))
            xit_f = const.tile([F, NI], f32)
            nc.sync.dma_start(xit_f[:], xit_d[:])
            w2_f = const.tile([K0, 2, OUT], f32)  # [:,0,:]=W2a  [0:64,1,:]=W2b
            nc.scalar.dma_start(w2_f[:], w2_d.rearrange("(two k) o -> k two o", two=2))
            b1c = const.tile([K0, 2], f32)  # [:,0]=b1[0:128]  [0:64,1]=b1[128:192]
            nc.sync.dma_start(b1c[:], b1_d.rearrange("(two k) one -> k (two one)", two=2))
            b2c = const.tile([OUT, 1], f32)
            nc.scalar.dma_start(b2c[:], b2_d[:])
            b2rep_f = const.tile([1, IB * N], f32)
            nc.sync.dma_start(b2rep_f[:], b2rep_d[:])

            xt_bf = const.tile([F, N], bf16)
            nc.vector.tensor_copy(xt_bf[:], xt_f[:])
            xit_bf = const.tile([F, NI], bf16)
            nc.vector.tensor_copy(xit_bf[:], xit_f[:])
            w1_bf = const.tile([F, 2, MID], bf16)
            nc.vector.tensor_copy(w1_bf[:], w1_f[:])
            w1a_bf = w1_bf[:, 0, :]
            w1b_bf = w1_bf[:, 1, :]
            w2a_bf = const.tile([K0, OUT], bf16)
            nc.vector.tensor_copy(w2a_bf[:], w2_f[:, 0, :])
            # W2b duplicated into both partition halves (pair-folded k1 matmuls)
            w2b_dup = const.tile([2 * K1, OUT], bf16)
            nc.gpsimd.tensor_copy(w2b_dup[0:K1, :], w2_f[0:K1, 1, :])
            nc.gpsimd.tensor_copy(w2b_dup[K1 : 2 * K1, :], w2_f[0:K1, 1, :])
            b2rep_bf = const.tile([1, IB * N], bf16)
            nc.gpsimd.tensor_copy(b2rep_bf[:], b2rep_f[:])
            ones_row = const.tile([1, OUT], bf16)
            nc.gpsimd.memset(ones_row[:], 1.0)
            ones_col = const.tile([K0, 1], bf16)
            nc.gpsimd.memset(ones_col[:], 1.0)

            with tc.tile_pool(name="psum_s", bufs=4, space="PSUM") as psum_s:
                # ---- AT = (xi @ W1a).T + b1 : [k, i] f32 ----
                ps_at0 = psum_s.tile([K0, NI], f32, tag="ps")
                nc.tensor.matmul(
                    ps_at0[:], w1a_bf[:, 0:K0], xit_bf[:], start=True, stop=True
                )
                at0 = const.tile([K0, NI], f32)
                nc.scalar.activation(
                    at0[:], ps_at0[:], Act.Identity, bias=b1c[:, 0:1]
                )
                ps_at1 = psum_s.tile([K1, NI], f32, tag="ps")
                nc.tensor.matmul(
                    ps_at1[:], w1a_bf[:, K0:MID], xit_bf[:], start=True, stop=True
                )
                at1 = const.tile([K1, NI], f32)
                nc.scalar.activation(
                    at1[:], ps_at1[:], Act.Identity, bias=b1c[0:K1, 1:2]
                )

                # AT1 columns interleaved into partition halves: [k1 | k1, pair]
                # (copies on GpSimd: SBUF->SBUF, keeps DVE free during setup)
                at1_prs = const.tile([2 * K1, NI // 2], f32)
                at1_pairs_view = at1.rearrange("k (p two) -> k two p", two=2)
                nc.vector.tensor_copy(at1_prs[0:K1, :], at1_pairs_view[:, 0, :])
                nc.vector.tensor_copy(at1_prs[K1 : 2 * K1, :], at1_pairs_view[:, 1, :])

                # ---- BT = (x @ W1b).T : [k, j] bf16 ----
                ps_bt0 = psum_s.tile([K0, N], f32, tag="ps")
                nc.tensor.matmul(
                    ps_bt0[:], w1b_bf[:, 0:K0], xt_bf[:], start=True, stop=True
                )
                bt0 = const.tile([K0, N], bf16)
                nc.vector.tensor_copy(bt0[:], ps_bt0[:])
                ps_bt1 = psum_s.tile([K1, N], f32, tag="ps")
                nc.tensor.matmul(
                    ps_bt1[:], w1b_bf[:, K0:MID], xt_bf[:], start=True, stop=True
                )
                # BT1 duplicated into both partition halves
                bt1_dup = const.tile([2 * K1, N], bf16)
                nc.vector.tensor_copy(bt1_dup[0:K1, :], ps_bt1[:])
                nc.vector.tensor_copy(bt1_dup[K1 : 2 * K1, :], ps_bt1[:])


            # ---- main loop: IB i's per 4-bank PSUM tile ----
            # DVE reduces are emitted one block late (software pipelining);
            # the last i of each block reduces on ACT's accumulator instead,
            # balancing DVE vs ACT.
            ot = const.tile([OUT, NI], f32)
            with tc.tile_pool(name="psum_m", bufs=3, space="PSUM") as psum_m:
                nblk = NI // IB
                pending = None  # (blk, vt, n_dve) awaiting DVE reduction

                def emit_reduces(blk, vt, n_dve):
                    for r in range(n_dve):
                        i = blk * IB + r
                        dump = work.tile([OUT, N], bf16, tag="dump", bufs=6)
                        nc.vector.tensor_scalar(
                            dump[:], vt[:, r * N : (r + 1) * N], 1.0, None,
                            Alu.mult, Alu.add, accum_out=ot[:, i : i + 1],
                        )

                # transposed blocks: [j, o]-oriented PSUM; the j-reduction
                # runs on PE (V^T stationary x ones), results land in psum_o
                # columns, extracted once near the end.
                famA = [b for b in range(nblk) if b % 4 == 3 and b != nblk - 1]
                famB = [b for b in range(nblk) if b % 8 == 5]
                famC = [b for b in range(nblk) if b % 16 == 9]
                tr_blks = sorted(famA + famB + famC)
                tr_col = {}
                col = 0
                for fam in (famA, famB, famC):
                    for b in fam:
                        tr_col[b] = col
                        col += IB
                psum_o = psum_m.tile([OUT, col], f32, tag="po", bufs=1)

                for blk in range(nblk):
                    transposed = blk in tr_col
                    pv = psum_m.tile([OUT, IB * N], f32, tag="pv")
                    if transposed:
                        # rank-1 bias fill: psum_tr[j, o-regions] = 1 (x) b2rep
                        for half in (0, 1):
                            nc.tensor.matmul(
                                pv[:, half * 512 : (half + 1) * 512],
                                ones_row[:], b2rep_bf[:, half * 512 : (half + 1) * 512],
                                start=True, stop=False, skip_group_check=True,
                            )
                    hts = []
                    for pp in range(IB // 2):
                        p = blk * (IB // 2) + pp
                        # k1 chunk for both i's of the pair in one op
                        ht1p = work.tile([2 * K1, N], bf16, tag="ht1", bufs=6)
                        nc.vector.tensor_scalar(
                            ht1p[:], bt1_dup[:], at1_prs[:, p : p + 1], 0.0,
                            Alu.add, Alu.max,
                        )
                        for q in (0, 1):
                            i = 2 * p + q
                            ht0 = work.tile([K0, N], bf16, tag="ht0", bufs=12)
                            nc.vector.tensor_scalar(
                                ht0[:], bt0[:], at0[:, i : i + 1], 0.0,
                                Alu.add, Alu.max,
                            )
                            r = 2 * pp + q
                            if transposed:
                                for jc in (0, 1):
                                    reg = pv[:, (2 * r + jc) * (N // 2) :
                                             (2 * r + jc + 1) * (N // 2)]
                                    nc.tensor.matmul(
                                        reg, ht0[:, jc * K0 : (jc + 1) * K0],
                                        w2a_bf[:], start=False, stop=False,
                                        skip_group_check=True,
                                    )
                                    nc.tensor.matmul(
                                        reg,
                                        ht1p[q * K1 : (q + 1) * K1,
                                             jc * K0 : (jc + 1) * K0],
                                        w2b_dup[q * K1 : (q + 1) * K1, :],
                                        start=False, stop=True,
                                        skip_group_check=True,
                                    )
                            else:
                                nc.tensor.matmul(
                                    pv[:, r * N : (r + 1) * N], w2a_bf[:], ht0[:],
                                    start=True, stop=False,
                                )
                                nc.tensor.matmul(
                                    pv[:, r * N : (r + 1) * N],
                                    w2b_dup[q * K1 : (q + 1) * K1, :],
                                    ht1p[q * K1 : (q + 1) * K1, :],
                                    start=False, stop=True,
                                )
                    vt = work.tile([OUT, IB * N], bf16, tag="vt", bufs=4)
                    if transposed:
                        nc.scalar.activation(vt[:], pv[:], Act.Tanh)
                        c0 = tr_col[blk]
                        for r in range(IB):
                            col = psum_o[:, c0 + r : c0 + r + 1]
                            for jc in (0, 1):
                                nc.tensor.matmul(
                                    col,
                                    vt[:, (2 * r + jc) * (N // 2) :
                                       (2 * r + jc + 1) * (N // 2)],
                                    ones_col[:, 0:1],
                                    start=(jc == 0), stop=(jc == 1),
                                )
                        n_dve = 0
                    else:
                        nc.scalar.activation(vt[:], pv[:], Act.Tanh, bias=b2c[:, 0:1])
                        n_dve = IB
                    if pending is not None:
                        emit_reduces(*pending)
                    pending = (blk, vt, n_dve)
                    if blk == 1:
                        # ---- diagonal terms VdT[o,i] = tanh(relu(AT+BdT)@W2+b2)
                        # (emitted late so setup isn't serialized on it)
                        ps_d0 = psum_m.tile([K0, NI], f32, tag="pd", bufs=1)
                        nc.tensor.matmul(
                            ps_d0[:], w1b_bf[:, 0:K0], xit_bf[:],
                            start=True, stop=True,
                        )
                        tmp_d0 = work.tile([K0, NI], f32, tag="tmpd")
                        nc.vector.tensor_tensor(
                            tmp_d0[:], ps_d0[:], at0[:], Alu.add
                        )
                        dd0 = work.tile([K0, NI], bf16, tag="dd")
                        nc.scalar.activation(dd0[:], tmp_d0[:], Act.Relu)
                        ps_d1 = psum_m.tile([K1, NI], f32, tag="pd", bufs=1)
                        nc.tensor.matmul(
                            ps_d1[:], w1b_bf[:, K0:MID], xit_bf[:],
                            start=True, stop=True,
                        )
                        tmp_d1 = work.tile([K1, NI], f32, tag="tmpd")
                        nc.vector.tensor_tensor(
                            tmp_d1[:], ps_d1[:], at1[:], Alu.add
                        )
                        dd1 = work.tile([K1, NI], bf16, tag="dd")
                        nc.scalar.activation(dd1[:], tmp_d1[:], Act.Relu)
                        ps_vd = psum_m.tile([OUT, NI], f32, tag="pd", bufs=1)
                        nc.tensor.matmul(
                            ps_vd[:], w2a_bf[:], dd0[:], start=True, stop=False
                        )
                        nc.tensor.matmul(
                            ps_vd[:], w2b_dup[0:K1, :], dd1[:],
                            start=False, stop=True,
                        )
                        vd = const.tile([OUT, NI], f32)
                        nc.scalar.activation(
                            vd[:], ps_vd[:], Act.Tanh, bias=b2c[:, 0:1]
                        )
                emit_reduces(*pending)

                # collect the PE-reduced columns into ot (vd handled by the
                # global final subtract); one strided op per family
                for fam, blkmod in ((famA, 4), (famB, 8), (famC, 16)):
                    if not fam:
                        continue
                    c0, c1 = tr_col[fam[0]], tr_col[fam[-1]] + IB
                    r = blkmod * IB
                    i0 = fam[0] * IB
                    view = ot.rearrange("o (g r) -> o g r", r=r)[
                        :, 0 : len(fam), i0 : i0 + IB
                    ]
                    nc.vector.tensor_scalar(
                        view, psum_o[:, c0:c1], 1.0, None, Alu.mult
                    )

            # ---- subtract diagonal, store (quarters) ----
            ot2 = const.tile([OUT, NI], f32)
            Q = NI // 4
            for qq in range(4):
                sl = slice(qq * Q, (qq + 1) * Q)
                nc.vector.tensor_tensor(
                    ot2[:, sl], ot[:, sl], vd[:, sl], Alu.subtract
                )
                nc.sync.dma_start(out_d[:, sl], ot2[:, sl])

    nc.compile()
    return nc


def _get_nc():
    if "nc" not in _CACHE:
        _CACHE["nc"] = _build()
    return _CACHE["nc"]


def make_in_maps(inp, W1, b1, W2, b2):
    """Shard FULL inputs into 8 per-core input maps."""
    inp = np.ascontiguousarray(inp, dtype=np.float32)
    w1 = np.ascontiguousarray(W1, dtype=np.float32)
    w2p = np.zeros((2 * K0, OUT), dtype=np.float32)
    w2p[:MID] = np.asarray(W2, dtype=np.float32)
    b1p = np.zeros((2 * K0, 1), dtype=np.float32)
    b1p[:MID, 0] = np.asarray(b1, dtype=np.float32).reshape(MID)
    b2c = np.ascontiguousarray(np.asarray(b2, dtype=np.float32).reshape(OUT, 1))
    b2rep = np.ascontiguousarray(
        np.tile(np.asarray(b2, dtype=np.float32).reshape(1, OUT), (1, IB * N // OUT))
    )
    in_maps = []
    for c in range(8):
        b, h = c // 2, c % 2
        xt = np.ascontiguousarray(inp[b].T)  # [F, N]
        xit = np.ascontiguousarray(inp[b, h * NI : (h + 1) * NI].T)  # [F, NI]
        in_maps.append(
            {"xt": xt, "xit": xit, "w1": w1, "w2": w2p, "b1": b1p, "b2": b2c,
             "b2rep": b2rep}
        )
    return in_maps


def assemble(core_outs):
    """[8 x [OUT, NI]] core outputs -> full [B, N, OUT]."""
    full = np.empty((B, N, OUT), dtype=np.float32)
    for c in range(8):
        b, h = c // 2, c % 2
        full[b, h * NI : (h + 1) * NI, :] = core_outs[c].T
    return full


def kernel(inp, W1, b1, W2, b2, _want_results=False, **run_kwargs):
    from concourse.bass_utils import run_bass_kernel_spmd

    nc = _get_nc()
    in_maps = make_in_maps(inp, W1, b1, W2, b2)
    res = run_bass_kernel_spmd(nc, in_maps, core_ids=list(range(8)), **run_kwargs)
    out = assemble([r["out"] for r in res.results])
    if _want_results:
        return out, res
    return out


# revision 54
# speedup vs baseline: 1.0282x; 1.0282x over previous
"""Trainium2 Bass kernel for pairwise-message GNN block.

reference math (B=4, N=256, F=128, MID=192):
    out[b,i] = sum_{j != i} tanh(relu(concat(x_i, x_j) @ W1 + b1) @ W2 + b2)

Decomposition used here:
    concat(x_i,x_j)@W1 = x_i@W1a + x_j@W1b          (W1a = W1[:F], W1b = W1[F:])
    out_i = sum_{j=0..N-1} tanh(relu(A_i + B_j)@W2 + b2) - tanh(relu(A_i + B_i)@W2 + b2)

Sharding: 8 cores = 4 batches x 2 halves of the source-node axis i.
Each core gets x[b].T, its own x[b, half].T, and replicated weights; computes
a [128 out_f, 128 i] transposed output tile; host reassembles + transposes.

Per-core dataflow (all matmuls bf16 with f32 PSUM accumulation):
    AT[k,i]  = (xiT.T @ W1a).T + b1      SBUF f32 [128+64, 128]
    BT[k,j]  = (xT.T @ W1b).T            SBUF bf16 [128, 256] + dup'd k1 rows
    loop over i (4 per PSUM tile):
      HT0      = relu(BT0 + AT0[:,i])    DVE tensor_scalar add+max -> bf16 (per i)
      HT1pair  = relu(BT1dup + AT1prs)   one op covers the k=128:192 chunk of TWO i's
      normal blocks (o-major PSUM):
        PSUM   = W2a.T@HT0 + W2b.T@HT1   PE -> [128 o, 256 j] per i
        VT     = tanh(PSUM + b2)         ACT, batched FD=1024
        OT[:,i]= sum_j VT                DVE tensor_scalar + accum_out
      transposed blocks (~40%, j-major PSUM; moves the j-reduction to PE):
        PSUM   = ones x b2rep            rank-1 bias fill
               += HT.T-chunks @ W2       PE -> [128 j-chunk, o] regions
        VT     = tanh(PSUM)              ACT FD=1024
        psum_o[:,c] = VT-chunks @ ones   PE column reduction, extracted at the
                                         end with one strided op per family
    OT -= VdT (diagonal terms, computed mid-loop); quarter-wise sub + DMA out.
"""

import numpy as np

B, N, F, MID, OUT = 4, 256, 128, 128 + 64, 128
NI = N // 2  # i's per core
K0, K1 = 128, MID - 128  # MID partition chunks
IB = 4  # i's per PSUM/activation block (2 PSUM banks)

_CACHE = {}


def _build():
    import concourse.bass as bass
    import concourse.tile as tile
    from concourse import bacc, mybir

    f32 = mybir.dt.float32
    bf16 = mybir.dt.bfloat16
    Alu = mybir.AluOpType
    Act = mybir.ActivationFunctionType

    nc = bacc.Bacc(
        "TRN2",
        target_bir_lowering=False,
        debug=False,
        enable_asserts=True,
        num_devices=8,
    )

    # matmul-facing inputs arrive pre-cast to bf16 from the host
    xt_d = nc.dram_tensor("xt", [F, N], bf16, kind="ExternalInput").ap()
    xit_d = nc.dram_tensor("xit", [F, NI], bf16, kind="ExternalInput").ap()
    # w1: [2F, MID] natural; w2/b1 zero-padded on host to 2*K0 rows so a
    # single folded-AP DMA loads both partition chunks.
    w1_d = nc.dram_tensor("w1", [2 * F, MID], bf16, kind="ExternalInput").ap()
    w2_d = nc.dram_tensor("w2", [2 * K0, OUT], bf16, kind="ExternalInput").ap()
    b1_d = nc.dram_tensor("b1", [2 * K0, 1], f32, kind="ExternalInput").ap()
    b2_d = nc.dram_tensor("b2", [OUT, 1], f32, kind="ExternalInput").ap()
    b2rep_d = nc.dram_tensor("b2rep", [1, IB * N], bf16, kind="ExternalInput").ap()
    out_d = nc.dram_tensor("out", [OUT, NI], f32, kind="ExternalOutput").ap()

    with tile.TileContext(nc) as tc:
        with (
            tc.tile_pool(name="const", bufs=1) as const,
            tc.tile_pool(name="work", bufs=4) as work,
        ):
            # ---- load (bf16 tensors DMA'd directly, no on-chip casts);
            # queue order by criticality: w1/xit gate the AT matmuls, xt the
            # BT matmuls ----
            w1_bf = const.tile([F, 2, MID], bf16)  # [:,0,:]=W1a  [:,1,:]=W1b
            nc.sync.dma_start(w1_bf[:], w1_d.rearrange("(two f) k -> f two k", two=2))
            xit_bf = const.tile([F, NI], bf16)
            nc.scalar.dma_start(xit_bf[:], xit_d[:])
            xt_bf = const.tile([F, N], bf16)
            nc.gpsimd.dma_start(xt_bf[:], xt_d[:])
            b1c = const.tile([K0, 2], f32)  # [:,0]=b1[0:128]  [0:64,1]=b1[128:192]
            nc.scalar.dma_start(b1c[:], b1_d.rearrange("(two k) one -> k (two one)", two=2))
            w2_bf = const.tile([K0, 2, OUT], bf16)  # [:,0,:]=W2a  [0:64,1,:]=W2b
            nc.sync.dma_start(w2_bf[:], w2_d.rearrange("(two k) o -> k two o", two=2))
            b2c = const.tile([OUT, 1], f32)
            nc.scalar.dma_start(b2c[:], b2_d[:])
            b2rep_bf = const.tile([1, IB * N], bf16)
            nc.scalar.dma_start(b2rep_bf[:], b2rep_d[:])

            w1a_bf = w1_bf[:, 0, :]
            w1b_bf = w1_bf[:, 1, :]
            w2a_bf = w2_bf[:, 0, :]
            # W2b duplicated into both partition halves (pair-folded k1 matmuls)
            w2b_dup = const.tile([2 * K1, OUT], bf16)
            nc.gpsimd.tensor_copy(w2b_dup[0:K1, :], w2_bf[0:K1, 1, :])
            nc.gpsimd.tensor_copy(w2b_dup[K1 : 2 * K1, :], w2_bf[0:K1, 1, :])
            ones_row = const.tile([1, OUT], bf16)
            nc.gpsimd.memset(ones_row[:], 1.0)
            ones_col = const.tile([K0, 1], bf16)
            nc.gpsimd.memset(ones_col[:], 1.0)

            # PE warmup: dummy matmuls on a zeroed tile while input DMAs are
            # in flight, so the HAM clock-gate opens before real work arrives
            warm = const.tile([K0, 512], bf16)
            nc.gpsimd.memset(warm[:], 0.0)
            # ACT warmup: trigger the tanh table load (~1.3us) during the DMA
            # wait instead of at the first real tanh mid-pipeline
            act_warm = const.tile([1, 8], f32)
            nc.scalar.activation(act_warm[0:1, 0:8], warm[0:1, 0:8], Act.Tanh)

            with tc.tile_pool(name="psum_s", bufs=4, space="PSUM") as psum_s:
                ps_w = psum_s.tile([K0, 512], f32, tag="warm", bufs=1)
                for _ in range(4):
                    nc.tensor.matmul(
                        ps_w[:], warm[:, 0:128], warm[:], start=True, stop=True
                    )

                # ---- AT = (xi @ W1a).T + b1 : [k, i] f32 ----
                ps_at0 = psum_s.tile([K0, NI], f32, tag="ps")
                nc.tensor.matmul(
                    ps_at0[:], w1a_bf[:, 0:K0], xit_bf[:], start=True, stop=True
                )
                at0 = const.tile([K0, NI], f32)
                nc.scalar.activation(
                    at0[:], ps_at0[:], Act.Identity, bias=b1c[:, 0:1]
                )
                ps_at1 = psum_s.tile([K1, NI], f32, tag="ps")
                nc.tensor.matmul(
                    ps_at1[:], w1a_bf[:, K0:MID], xit_bf[:], start=True, stop=True
                )
                at1 = const.tile([K1, NI], f32)
                nc.scalar.activation(
                    at1[:], ps_at1[:], Act.Identity, bias=b1c[0:K1, 1:2]
                )

                # AT1 columns interleaved into partition halves: [k1 | k1, pair]
                # built from PSUM directly with the b1 add fused
                at1_prs = const.tile([2 * K1, NI // 2], f32)
                ps_at1_view = ps_at1.rearrange("k (p two) -> k two p", two=2)
                nc.vector.tensor_scalar(
                    at1_prs[0:K1, :], ps_at1_view[:, 0, :], b1c[0:K1, 1:2], None,
                    Alu.add,
                )
                nc.vector.tensor_scalar(
                    at1_prs[K1 : 2 * K1, :], ps_at1_view[:, 1, :], b1c[0:K1, 1:2],
                    None, Alu.add,
                )

                # ---- BT = (x @ W1b).T : [k, j] bf16 ----
                ps_bt0 = psum_s.tile([K0, N], f32, tag="ps")
                nc.tensor.matmul(
                    ps_bt0[:], w1b_bf[:, 0:K0], xt_bf[:], start=True, stop=True
                )
                bt0 = const.tile([K0, N], bf16)
                nc.vector.tensor_copy(bt0[:], ps_bt0[:])
                ps_bt1 = psum_s.tile([K1, N], f32, tag="ps")
                nc.tensor.matmul(
                    ps_bt1[:], w1b_bf[:, K0:MID], xt_bf[:], start=True, stop=True
                )
                # BT1 duplicated into both partition halves
                bt1_dup = const.tile([2 * K1, N], bf16)
                nc.scalar.activation(bt1_dup[0:K1, :], ps_bt1[:], Act.Copy)
                nc.scalar.activation(bt1_dup[K1 : 2 * K1, :], ps_bt1[:], Act.Copy)


            # ---- main loop: IB i's per 4-bank PSUM tile ----
            # DVE reduces are emitted one block late (software pipelining);
            # the last i of each block reduces on ACT's accumulator instead,
            # balancing DVE vs ACT.
            ot = const.tile([OUT, NI], f32)
            with tc.tile_pool(name="psum_m", bufs=3, space="PSUM") as psum_m:
                nblk = NI // IB - 1  # last block handled as two half blocks
                pendings = []  # (i0, vt, n_dve) awaiting DVE reduction (lag 2)

                def emit_reduces(i0, vt, n_dve):
                    for r in range(n_dve):
                        i = i0 + r
                        dump = work.tile([OUT, N], bf16, tag="dump", bufs=6)
                        nc.vector.tensor_scalar(
                            dump[:], vt[:, r * N : (r + 1) * N], 1.0, None,
                            Alu.mult, Alu.add, accum_out=ot[:, i : i + 1],
                        )

                # transposed blocks: [j, o]-oriented PSUM; the j-reduction
                # runs on PE (V^T stationary x ones), results land in psum_o
                # columns, extracted once near the end.
                famA = [b for b in range(nblk) if b % 4 == 3]
                famB = [b for b in range(nblk) if b % 8 == 5]
                famC = [b for b in range(nblk) if b % 16 == 9]
                famD = []
                tr_blks = sorted(famA + famB + famC + famD)
                tr_col = {}
                col = 0
                for fam in (famA, famB, famC, famD):
                    for b in fam:
                        tr_col[b] = col
                        col += IB
                psum_o = psum_m.tile([OUT, col], f32, tag="po", bufs=1)

                for blk in range(nblk):
                    transposed = blk in tr_col
                    pv = psum_m.tile([OUT, IB * N], f32, tag="pv")
                    if transposed:
                        # rank-1 bias fill: psum_tr[j, o-regions] = 1 (x) b2rep
                        for half in (0, 1):
                            nc.tensor.matmul(
                                pv[:, half * 512 : (half + 1) * 512],
                                ones_row[:], b2rep_bf[:, half * 512 : (half + 1) * 512],
                                start=True, stop=False, skip_group_check=True,
                            )
                    for pp in range(IB // 2):
                        p = blk * (IB // 2) + pp
                        # k1 chunk for both i's of the pair in one op
                        ht1p = work.tile([2 * K1, N], bf16, tag="ht1", bufs=6)
                        nc.vector.tensor_scalar(
                            ht1p[:], bt1_dup[:], at1_prs[:, p : p + 1], 0.0,
                            Alu.add, Alu.max,
                        )
                        for q in (0, 1):
                            i = 2 * p + q
                            ht0 = work.tile([K0, N], bf16, tag="ht0", bufs=12)
                            nc.vector.tensor_scalar(
                                ht0[:], bt0[:], at0[:, i : i + 1], 0.0,
                                Alu.add, Alu.max,
                            )
                            r = 2 * pp + q
                            if transposed:
                                for jc in (0, 1):
                                    reg = pv[:, (2 * r + jc) * (N // 2) :
                                             (2 * r + jc + 1) * (N // 2)]
                                    nc.tensor.matmul(
                                        reg, ht0[:, jc * K0 : (jc + 1) * K0],
                                        w2a_bf[:], start=False, stop=False,
                                        skip_group_check=True,
                                    )
                                    nc.tensor.matmul(
                                        reg,
                                        ht1p[q * K1 : (q + 1) * K1,
                                             jc * K0 : (jc + 1) * K0],
                                        w2b_dup[q * K1 : (q + 1) * K1, :],
                                        start=False, stop=True,
                                        skip_group_check=True,
                                    )
                            else:
                                nc.tensor.matmul(
                                    pv[:, r * N : (r + 1) * N], w2a_bf[:], ht0[:],
                                    start=True, stop=False,
                                )
                                nc.tensor.matmul(
                                    pv[:, r * N : (r + 1) * N],
                                    w2b_dup[q * K1 : (q + 1) * K1, :],
                                    ht1p[q * K1 : (q + 1) * K1, :],
                                    start=False, stop=True,
                                )
                    vt = work.tile([OUT, IB * N], bf16, tag="vt", bufs=4)
                    if transposed:
                        nc.scalar.activation(vt[:], pv[:], Act.Tanh)
                        c0 = tr_col[blk]
                        for r in range(IB):
                            col = psum_o[:, c0 + r : c0 + r + 1]
                            for jc in (0, 1):
                                nc.tensor.matmul(
                                    col,
                                    vt[:, (2 * r + jc) * (N // 2) :
                                       (2 * r + jc + 1) * (N // 2)],
                                    ones_col[:, 0:1],
                                    start=(jc == 0), stop=(jc == 1),
                                )
                        n_dve = 0
                    else:
                        nc.scalar.activation(vt[:], pv[:], Act.Tanh, bias=b2c[:, 0:1])
                        n_dve = IB
                    pendings.append((blk * IB, vt, n_dve))
                    if len(pendings) > 2:
                        emit_reduces(*pendings.pop(0))
                    if blk == 1:
                        # ---- diagonal terms VdT[o,i] = tanh(relu(AT+BdT)@W2+b2)
                        # (emitted late so setup isn't serialized on it)
                        ps_d0 = psum_m.tile([K0, NI], f32, tag="pd", bufs=1)
                        nc.tensor.matmul(
                            ps_d0[:], w1b_bf[:, 0:K0], xit_bf[:],
                            start=True, stop=True,
                        )
                        tmp_d0 = work.tile([K0, NI], f32, tag="tmpd")
                        nc.vector.tensor_tensor(
                            tmp_d0[:], ps_d0[:], at0[:], Alu.add
                        )
                        dd0 = work.tile([K0, NI], bf16, tag="dd")
                        nc.scalar.activation(dd0[:], tmp_d0[:], Act.Relu)
                        ps_d1 = psum_m.tile([K1, NI], f32, tag="pd", bufs=1)
                        nc.tensor.matmul(
                            ps_d1[:], w1b_bf[:, K0:MID], xit_bf[:],
                            start=True, stop=True,
                        )
                        tmp_d1 = work.tile([K1, NI], f32, tag="tmpd")
                        nc.vector.tensor_tensor(
                            tmp_d1[:], ps_d1[:], at1[:], Alu.add
                        )
                        dd1 = work.tile([K1, NI], bf16, tag="dd")
                        nc.scalar.activation(dd1[:], tmp_d1[:], Act.Relu)
                        ps_vd = psum_m.tile([OUT, NI], f32, tag="pd", bufs=1)
                        nc.tensor.matmul(
                            ps_vd[:], w2a_bf[:], dd0[:], start=True, stop=False
                        )
                        nc.tensor.matmul(
                            ps_vd[:], w2b_dup[0:K1, :], dd1[:],
                            start=False, stop=True,
                        )
                        vd = const.tile([OUT, NI], f32)
                        nc.scalar.activation(
                            vd[:], ps_vd[:], Act.Tanh, bias=b2c[:, 0:1]
                        )
                # two half-size tail blocks shorten the drain chain
                HB = IB // 2
                for tb in range(2):
                    i0 = nblk * IB + tb * HB
                    pvt = psum_m.tile([OUT, HB * N], f32, tag="pv")
                    p = i0 // 2
                    ht1p = work.tile([2 * K1, N], bf16, tag="ht1", bufs=6)
                    nc.vector.tensor_scalar(
                        ht1p[:], bt1_dup[:], at1_prs[:, p : p + 1], 0.0,
                        Alu.add, Alu.max,
                    )
                    for q in (0, 1):
                        i = i0 + q
                        ht0 = work.tile([K0, N], bf16, tag="ht0", bufs=12)
                        nc.vector.tensor_scalar(
                            ht0[:], bt0[:], at0[:, i : i + 1], 0.0,
                            Alu.add, Alu.max,
                        )
                        nc.tensor.matmul(
                            pvt[:, q * N : (q + 1) * N], w2a_bf[:], ht0[:],
                            start=True, stop=False,
                        )
                        nc.tensor.matmul(
                            pvt[:, q * N : (q + 1) * N],
                            w2b_dup[q * K1 : (q + 1) * K1, :],
                            ht1p[q * K1 : (q + 1) * K1, :],
                            start=False, stop=True,
                        )
                    vtt = work.tile([OUT, HB * N], bf16, tag="vt", bufs=4)
                    nc.scalar.activation(vtt[:], pvt[:], Act.Tanh, bias=b2c[:, 0:1])
                    pendings.append((i0, vtt, HB))
                    if len(pendings) > 2:
                        emit_reduces(*pendings.pop(0))
                for p_ in pendings:
                    emit_reduces(*p_)

                # collect the PE-reduced columns into ot (vd handled by the
                # global final subtract); one strided op per family
                for fam, blkmod in ((famA, 4), (famB, 8), (famC, 16), (famD, 32)):
                    if not fam:
                        continue
                    c0, c1 = tr_col[fam[0]], tr_col[fam[-1]] + IB
                    r = blkmod * IB
                    i0 = fam[0] * IB
                    view = ot.rearrange("o (g r) -> o g r", r=r)[
                        :, 0 : len(fam), i0 : i0 + IB
                    ]
                    nc.vector.tensor_scalar(
                        view, psum_o[:, c0:c1], 1.0, None, Alu.mult
                    )

            # ---- subtract diagonal, store (quarters) ----
            ot2 = const.tile([OUT, NI], f32)
            Q = NI // 4
            for qq in range(4):
                sl = slice(qq * Q, (qq + 1) * Q)
                nc.vector.tensor_tensor(
                    ot2[:, sl], ot[:, sl], vd[:, sl], Alu.subtract
                )
                nc.sync.dma_start(out_d[:, sl], ot2[:, sl])

    nc.compile()
    return nc


def _get_nc():
    if "nc" not in _CACHE:
        _CACHE["nc"] = _build()
    return _CACHE["nc"]


def make_in_maps(inp, W1, b1, W2, b2):
    """Shard FULL inputs into 8 per-core input maps (matmul operands pre-cast
    to bf16 on the host)."""
    import ml_dtypes

    bf = ml_dtypes.bfloat16
    inp = np.ascontiguousarray(inp, dtype=np.float32)
    w1 = np.ascontiguousarray(np.asarray(W1, dtype=np.float32).astype(bf))
    w2p = np.zeros((2 * K0, OUT), dtype=bf)
    w2p[:MID] = np.asarray(W2, dtype=np.float32).astype(bf)
    b1p = np.zeros((2 * K0, 1), dtype=np.float32)
    b1p[:MID, 0] = np.asarray(b1, dtype=np.float32).reshape(MID)
    b2c = np.ascontiguousarray(np.asarray(b2, dtype=np.float32).reshape(OUT, 1))
    b2rep = np.ascontiguousarray(
        np.tile(
            np.asarray(b2, dtype=np.float32).reshape(1, OUT), (1, IB * N // OUT)
        ).astype(bf)
    )
    in_maps = []
    for c in range(8):
        b, h = c // 2, c % 2
        xt = np.ascontiguousarray(inp[b].T.astype(bf))  # [F, N]
        xit = np.ascontiguousarray(
            inp[b, h * NI : (h + 1) * NI].T.astype(bf)
        )  # [F, NI]
        in_maps.append(
            {"xt": xt, "xit": xit, "w1": w1, "w2": w2p, "b1": b1p, "b2": b2c,
             "b2rep": b2rep}
        )
    return in_maps


def assemble(core_outs):
    """[8 x [OUT, NI]] core outputs -> full [B, N, OUT]."""
    full = np.empty((B, N, OUT), dtype=np.float32)
    for c in range(8):
        b, h = c // 2, c % 2
        full[b, h * NI : (h + 1) * NI, :] = core_outs[c].T
    return full


def kernel(inp, W1, b1, W2, b2, _want_results=False, **run_kwargs):
    from concourse.bass_utils import run_bass_kernel_spmd

    nc = _get_nc()
    in_maps = make_in_maps(inp, W1, b1, W2, b2)
    res = run_bass_kernel_spmd(nc, in_maps, core_ids=list(range(8)), **run_kwargs)
    out = assemble([r["out"] for r in res.results])
    if _want_results:
        return out, res
    return out
